# revision 6
# baseline (speedup 1.0000x reference)
"""Trainium2 Bass kernel for nn_LowRankRNN.

Math:  h_{t} = 0.9*h_{t-1} + 0.1*tanh(h_{t-1}) @ (n m^T) + 0.1*xp_t,
       xp_t = x_t @ I^T   (per batch row; sequential over t, B rows independent)

Strategy:
  - Data-parallel over batch: 8 cores x 4 rows each.
  - Time-sharding within each core: C chunks of L=T/C steps; each chunk
    starts W warmup steps early from h=0 (zero-padded x makes chunk 0 exact).
    The recurrence Jacobian has spectral radius ~0.91, so the warmup error
    after W=192 steps is ~3e-8 relative -- below fp32 roundoff.
  - Per serial slot tau, all C chunks advance together: state tile
    [128 partitions = h%128, F = (hg, c, b)] with hg = h//128 (4 groups),
    c = chunk, b = local batch row.
  - Per slot: ACT tanh -> 4 PE matmuls (contract H: v = tanh(h) @ n, rank 2)
    -> DVE copy psum->sbuf -> 4 PE matmuls (expand: g = v @ (0.1 m)^T),
    accumulating onto a PSUM bank pre-staged with e = 0.1*x_t@I^T by bulk
    matmuls -> one fused DVE scalar_tensor_tensor: h' = 0.9*h + psum(e+g).
"""

import sys

sys.path.insert(0, "/opt/trn_rl_repo")

import numpy as np

from concourse import bass, bacc, mybir
from concourse.tile import TileContext
from concourse.bass_utils import run_bass_kernel_spmd

# ---- problem constants (hardcoded; kernel.py must be self-contained) ----
B, T, D, H, R = 32, 2048, 128, 512, 2
ALPHA = 0.1
DECAY = 1.0 - ALPHA  # 0.9
NCORES = 8
BL = B // NCORES  # 4 batch rows per core

# ---- kernel tuning parameters (defaults; overridable via set_config) ----
C = 16            # time chunks per core
W = 192           # warmup steps (error ~3e-8, << fp32 noise)
HG = H // 128     # 4 h-groups
PSUM_COLS = 512
F32 = mybir.dt.float32


def _derived():
    L = T // C
    S = L + W
    CB = C * BL
    F = HG * CB
    SL = max(1, PSUM_COLS // F)
    TPAD = T + W
    return L, S, CB, F, SL, TPAD


def set_config(c=None, w=None):
    global C, W, _NC_CACHE
    if c is not None:
        C = c
    if w is not None:
        W = w
    _NC_CACHE = None


def build_nc():
    L, S, CB, F, SL, TPAD = _derived()
    nc = bacc.Bacc()

    xt = nc.declare_dram_parameter("xt", [128, TPAD * BL], F32, isOutput=False)
    isb = nc.declare_dram_parameter("isb", [128, H], F32, isOutput=False)
    msb = nc.declare_dram_parameter("msb", [2, H], F32, isOutput=False)
    nsb = nc.declare_dram_parameter("nsb", [128, HG * R], F32, isOutput=False)
    outk = nc.declare_dram_parameter("outk", [128, L * F], F32, isOutput=True)

    AF = mybir.ActivationFunctionType
    OP = mybir.AluOpType

    with TileContext(nc) as tc:
        with (
            tc.tile_pool(name="const", bufs=1) as constp,
            tc.tile_pool(name="thp", bufs=3) as thp,
            tc.tile_pool(name="vtp", bufs=3) as vtp,
            tc.tile_pool(name="hstate", bufs=8) as hp,
            tc.tile_pool(name="egp", bufs=6, space="PSUM") as egp,
            tc.tile_pool(name="pvp", bufs=2, space="PSUM") as pvp,
        ):
            xt_sb = constp.tile([128, TPAD * BL], F32, tag="xt")
            isb_sb = constp.tile([128, H], F32, tag="isb")
            msb_sb = constp.tile([2, H], F32, tag="msb")
            nsb_sb = constp.tile([128, HG * R], F32, tag="nsb")
            nc.sync.dma_start(out=xt_sb[:, :], in_=xt[:, :])
            nc.sync.dma_start(out=isb_sb[:, :], in_=isb[:, :])
            nc.sync.dma_start(out=msb_sb[:, :], in_=msb[:, :])
            nc.sync.dma_start(out=nsb_sb[:, :], in_=nsb[:, :])
            # Collapse the many per-DMA-queue semaphores into one barrier so
            # downstream matmuls don't exceed the ISA sync-wait slot limit.
            tc.strict_bb_all_engine_barrier()

            xt_pitch = xt_sb.ap[0][0]  # per-partition pitch in elements

            s_prev = hp.tile([128, F], F32, tag="h")
            nc.vector.memset(s_prev[:, :], 0.0)

            eg = None
            egr = None
            for tau in range(S):
                sl = tau % SL
                if sl == 0:
                    # stage e = 0.1 * x_t @ I^T for the next SL slots into a
                    # fresh psum bank; one matmul per h-group, free dims
                    # (slot, chunk, batch) with overlapping chunk windows.
                    eg = egp.tile([128, PSUM_COLS], F32, tag="eg")
                    egr = eg.rearrange(
                        "p (s g c b) -> p s g c b", s=SL, g=HG, c=C, b=BL
                    )
                    rhs = bass.AP(
                        xt_sb.tensor,
                        xt_sb.offset + tau * BL,
                        [[xt_pitch, 128], [BL, SL], [L * BL, C], [1, BL]],
                    )
                    for hg in range(HG):
                        # start=True clears the whole psum bank, so only the
                        # first matmul of the bank may set it.
                        nc.tensor.matmul(
                            egr[:, :, hg, :, :],
                            isb_sb[:, hg * 128 : (hg + 1) * 128],
                            rhs,
                            start=(hg == 0),
                            stop=False,
                        )

                # th = tanh(h)
                th = thp.tile([128, F], F32, tag="th")
                nc.scalar.activation(th[:, :], s_prev[:, :], AF.Tanh)

                # v = tanh(h) @ n : contract H over 4 groups into psum [2, CB]
                pv = pvp.tile([2, CB], F32, tag="pv")
                for hg in range(HG):
                    nc.tensor.matmul(
                        pv[:, :],
                        nsb_sb[:, hg * R : (hg + 1) * R],
                        th[:, hg * CB : (hg + 1) * CB],
                        start=(hg == 0),
                        stop=(hg == HG - 1),
                    )

                vt = vtp.tile([2, CB], F32, tag="vt")
                nc.vector.tensor_copy(vt[:, :], pv[:, :])

                # g = v @ (0.1 m)^T accumulated onto the staged e bank
                for hg in range(HG):
                    nc.tensor.matmul(
                        egr[:, sl, hg, :, :],
                        msb_sb[:, hg * 128 : (hg + 1) * 128],
                        vt[:, :],
                        start=False,
                        stop=True,
                    )

                # h' = 0.9*h + (e + g)
                s_new = hp.tile([128, F], F32, tag="h")
                nc.vector.scalar_tensor_tensor(
                    s_new[:, :],
                    s_prev[:, :],
                    DECAY,
                    eg[:, sl * F : (sl + 1) * F],
                    OP.mult,
                    OP.add,
                )

                if tau >= W:
                    j = tau - W
                    nc.sync.dma_start(
                        out=outk[:, j * F : (j + 1) * F], in_=s_new[:, :]
                    )
                s_prev = s_new

    nc.finalize()
    return nc


_NC_CACHE = None


def _get_nc():
    global _NC_CACHE
    if _NC_CACHE is None:
        _NC_CACHE = build_nc()
    return _NC_CACHE


def prepare_inputs(x, m, n, I):
    """Build the per-core input maps (host-side layout transforms)."""
    L, S, CB, F, SL, TPAD = _derived()
    x = np.asarray(x, dtype=np.float32)
    m = np.asarray(m, dtype=np.float32)
    n = np.asarray(n, dtype=np.float32)
    I = np.asarray(I, dtype=np.float32)

    isb = np.ascontiguousarray((ALPHA * I).T)  # [128, H]
    msb = np.ascontiguousarray((ALPHA * m).T)  # [2, H]
    nsb = np.ascontiguousarray(
        n.reshape(HG, 128, R).transpose(1, 0, 2).reshape(128, HG * R)
    )  # [128, (hg, r)]

    in_maps = []
    for k in range(NCORES):
        xs = x[k * BL : (k + 1) * BL]          # [BL, T, D]
        xtc = xs.transpose(2, 1, 0)            # [D, T, BL]
        xpad = np.zeros((128, TPAD, BL), np.float32)
        xpad[:, W:, :] = xtc
        in_maps.append(
            {
                "xt": np.ascontiguousarray(xpad.reshape(128, TPAD * BL)),
                "isb": isb,
                "msb": msb,
                "nsb": nsb,
            }
        )
    return in_maps


def assemble_output(results):
    L, S, CB, F, SL, TPAD = _derived()
    out = np.empty((B, T, H), np.float32)
    for k in range(NCORES):
        arr = results[k]["outk"].reshape(128, L, HG, C, BL)
        # h[b, c*L + j, hg*128 + p] = arr[p, j, hg, c, b]
        shard = arr.transpose(4, 3, 1, 2, 0).reshape(BL, T, H)
        out[k * BL : (k + 1) * BL] = shard
    return out


def kernel(x, m, n, I, _trace=False):
    nc = _get_nc()
    in_maps = prepare_inputs(x, m, n, I)
    res = run_bass_kernel_spmd(nc, in_maps, list(range(NCORES)), trace=_trace)
    out = assemble_output(res.results)
    if _trace:
        kernel.last_results = res
    return out


# revision 7
# speedup vs baseline: 1.4874x; 1.4874x over previous
"""Trainium2 Bass kernel for nn_LowRankRNN.

Math:  h_{t} = 0.9*h_{t-1} + 0.1*tanh(h_{t-1}) @ (n m^T) + 0.1*xp_t,
       xp_t = x_t @ I^T   (per batch row; sequential over t, B rows independent)

Strategy:
  - Data-parallel over batch: 8 cores x 4 rows each.
  - Time-sharding within each core: C chunks of L=T/C steps; each chunk
    starts W warmup steps early from h=0 (zero-padded x makes chunk 0 exact).
    The recurrence Jacobian has spectral radius ~0.91, so the warmup error
    after W=192 steps is ~3e-8 relative -- below fp32 roundoff.
  - Per serial slot tau, all C chunks advance together: state tile
    [128 partitions = h%128, F = (hg, c, b)] with hg = h//128 (4 groups),
    c = chunk, b = local batch row.
  - Per slot: ACT tanh -> 4 PE matmuls (contract H: v = tanh(h) @ n, rank 2)
    -> DVE copy psum->sbuf -> 4 PE matmuls (expand: g = v @ (0.1 m)^T),
    accumulating onto a PSUM bank pre-staged with e = 0.1*x_t@I^T by bulk
    matmuls -> one fused DVE scalar_tensor_tensor: h' = 0.9*h + psum(e+g).
"""

import sys

sys.path.insert(0, "/opt/trn_rl_repo")

import numpy as np

from concourse import bass, bacc, mybir
from concourse.tile import TileContext
from concourse.bass_utils import run_bass_kernel_spmd

# ---- problem constants (hardcoded; kernel.py must be self-contained) ----
B, T, D, H, R = 32, 2048, 128, 512, 2
ALPHA = 0.1
DECAY = 1.0 - ALPHA  # 0.9
NCORES = 8
BL = B // NCORES  # 4 batch rows per core

# ---- kernel tuning parameters (defaults; overridable via set_config) ----
C = 16            # time chunks per core
W = 192           # warmup steps (error ~3e-8, << fp32 noise)
HG = H // 128     # 4 h-groups
PSUM_COLS = 512
F32 = mybir.dt.float32
BF16 = mybir.dt.bfloat16


def _derived():
    L = T // C
    S = L + W
    CB = C * BL
    F = HG * CB
    SL = max(1, PSUM_COLS // F)
    TPAD = T + W
    return L, S, CB, F, SL, TPAD


def set_config(c=None, w=None):
    global C, W, _NC_CACHE
    if c is not None:
        C = c
    if w is not None:
        W = w
    _NC_CACHE = None


def build_nc():
    L, S, CB, F, SL, TPAD = _derived()
    nc = bacc.Bacc()

    xt = nc.declare_dram_parameter("xt", [128, TPAD * BL], F32, isOutput=False)
    isb = nc.declare_dram_parameter("isb", [128, H], F32, isOutput=False)
    msb = nc.declare_dram_parameter("msb", [2, H], F32, isOutput=False)
    nsb = nc.declare_dram_parameter("nsb", [128, HG * R], F32, isOutput=False)
    npa = nc.declare_dram_parameter("npa", [128, HG * 8], BF16, isOutput=False)
    npb = nc.declare_dram_parameter("npb", [128, HG * 8], BF16, isOutput=False)
    mpa = nc.declare_dram_parameter("mpa", [8, H], BF16, isOutput=False)
    mpb = nc.declare_dram_parameter("mpb", [8, H], BF16, isOutput=False)
    outk = nc.declare_dram_parameter("outk", [128, L * F], F32, isOutput=True)

    AF = mybir.ActivationFunctionType
    OP = mybir.AluOpType

    with TileContext(nc) as tc:
        with (
            tc.tile_pool(name="const", bufs=1) as constp,
            tc.tile_pool(name="thp", bufs=3) as thp,
            tc.tile_pool(name="vtp", bufs=3) as vtp,
            tc.tile_pool(name="hstate", bufs=8) as hp,
            tc.tile_pool(name="egp", bufs=6, space="PSUM") as egp,
            tc.tile_pool(name="pvp", bufs=2, space="PSUM") as pvp,
        ):
            xt_sb = constp.tile([128, TPAD * BL], F32, tag="xt")
            isb_sb = constp.tile([128, H], F32, tag="isb")
            msb_sb = constp.tile([2, H], F32, tag="msb")
            nsb_sb = constp.tile([128, HG * R], F32, tag="nsb")
            npa_sb = constp.tile([128, HG * 8], BF16, tag="npa")
            npb_sb = constp.tile([128, HG * 8], BF16, tag="npb")
            mpa_sb = constp.tile([8, H], BF16, tag="mpa")
            mpb_sb = constp.tile([8, H], BF16, tag="mpb")
            nc.sync.dma_start(out=xt_sb[:, :], in_=xt[:, :])
            nc.sync.dma_start(out=isb_sb[:, :], in_=isb[:, :])
            nc.sync.dma_start(out=msb_sb[:, :], in_=msb[:, :])
            nc.sync.dma_start(out=nsb_sb[:, :], in_=nsb[:, :])
            nc.sync.dma_start(out=npa_sb[:, :], in_=npa[:, :])
            nc.sync.dma_start(out=npb_sb[:, :], in_=npb[:, :])
            nc.sync.dma_start(out=mpa_sb[:, :], in_=mpa[:, :])
            nc.sync.dma_start(out=mpb_sb[:, :], in_=mpb[:, :])
            # Collapse the many per-DMA-queue semaphores into one barrier so
            # downstream matmuls don't exceed the ISA sync-wait slot limit.
            tc.strict_bb_all_engine_barrier()

            xt_pitch = xt_sb.ap[0][0]  # per-partition pitch in elements

            s_prev = hp.tile([128, F], F32, tag="h")
            nc.vector.memset(s_prev[:, :], 0.0)

            eg = None
            egr = None
            for tau in range(S):
                sl = tau % SL
                if sl == 0:
                    # stage e = 0.1 * x_t @ I^T for the next SL slots into a
                    # fresh psum bank; one matmul per h-group, free dims
                    # (slot, chunk, batch) with overlapping chunk windows.
                    eg = egp.tile([128, PSUM_COLS], F32, tag="eg")
                    egr = eg.rearrange(
                        "p (s g c b) -> p s g c b", s=SL, g=HG, c=C, b=BL
                    )
                    rhs = bass.AP(
                        xt_sb.tensor,
                        xt_sb.offset + tau * BL,
                        [[xt_pitch, 128], [BL, SL], [L * BL, C], [1, BL]],
                    )
                    for hg in range(HG):
                        # start=True clears the whole psum bank, so only the
                        # first matmul of the bank may set it.
                        nc.tensor.matmul(
                            egr[:, :, hg, :, :],
                            isb_sb[:, hg * 128 : (hg + 1) * 128],
                            rhs,
                            start=(hg == 0),
                            stop=False,
                        )

                # th = tanh(h)
                th = thp.tile([128, F], F32, tag="th")
                nc.scalar.activation(th[:, :], s_prev[:, :], AF.Tanh)
                th_hi = thp.tile([128, F], BF16, tag="th_hi")
                nc.vector.tensor_copy(th_hi[:, :], th[:, :])
                th_lo = thp.tile([128, F], BF16, tag="th_lo")
                nc.vector.tensor_tensor(
                    th_lo[:, :], th[:, :], th_hi[:, :], OP.subtract
                )

                # v = tanh(h) @ n : contract H over 4 groups into psum [2, CB]
                pv = pvp.tile([8, CB], F32, tag="pv")
                for hg in range(HG):
                    nc.tensor.matmul(
                        pv[:, :],
                        npa_sb[:, hg * 8 : (hg + 1) * 8],
                        th_hi[:, hg * CB : (hg + 1) * CB],
                        start=(hg == 0),
                        stop=False,
                    )
                for hg in range(HG):
                    nc.tensor.matmul(
                        pv[:, :],
                        npb_sb[:, hg * 8 : (hg + 1) * 8],
                        th_lo[:, hg * CB : (hg + 1) * CB],
                        start=False,
                        stop=(hg == HG - 1),
                    )

                vt_hi = vtp.tile([8, CB], BF16, tag="vt_hi")
                nc.vector.tensor_copy(vt_hi[:, :], pv[:, :])
                vt_lo = vtp.tile([8, CB], BF16, tag="vt_lo")
                nc.vector.tensor_tensor(
                    vt_lo[:, :], pv[:, :], vt_hi[:, :], OP.subtract
                )

                # g = v @ (0.1 m)^T accumulated onto the staged e bank
                for hg in range(HG):
                    nc.tensor.matmul(
                        egr[:, sl, hg, :, :],
                        mpa_sb[:, hg * 128 : (hg + 1) * 128],
                        vt_hi[:, :],
                        start=False,
                        stop=False,
                    )
                    nc.tensor.matmul(
                        egr[:, sl, hg, :, :],
                        mpb_sb[:, hg * 128 : (hg + 1) * 128],
                        vt_lo[:, :],
                        start=False,
                        stop=True,
                    )

                # h' = 0.9*h + (e + g)
                s_new = hp.tile([128, F], F32, tag="h")
                nc.vector.scalar_tensor_tensor(
                    s_new[:, :],
                    s_prev[:, :],
                    DECAY,
                    eg[:, sl * F : (sl + 1) * F],
                    OP.mult,
                    OP.add,
                )

                if tau >= W:
                    j = tau - W
                    nc.sync.dma_start(
                        out=outk[:, j * F : (j + 1) * F], in_=s_new[:, :]
                    )
                s_prev = s_new

    nc.finalize()
    return nc


_NC_CACHE = None


def _get_nc():
    global _NC_CACHE
    if _NC_CACHE is None:
        _NC_CACHE = build_nc()
    return _NC_CACHE


def prepare_inputs(x, m, n, I):
    """Build the per-core input maps (host-side layout transforms)."""
    L, S, CB, F, SL, TPAD = _derived()
    x = np.asarray(x, dtype=np.float32)
    m = np.asarray(m, dtype=np.float32)
    n = np.asarray(n, dtype=np.float32)
    I = np.asarray(I, dtype=np.float32)

    isb = np.ascontiguousarray((ALPHA * I).T)  # [128, H]
    msb = np.ascontiguousarray((ALPHA * m).T)  # [2, H]
    nsb = np.ascontiguousarray(
        n.reshape(HG, 128, R).transpose(1, 0, 2).reshape(128, HG * R)
    )  # [128, (hg, r)]

    import ml_dtypes
    bf = ml_dtypes.bfloat16
    n_hi = n.astype(bf).astype(np.float32)
    n_lo = (n - n_hi).astype(bf).astype(np.float32)
    m01 = (ALPHA * m).astype(np.float32)
    m_hi = m01.astype(bf).astype(np.float32)
    m_lo = (m01 - m_hi).astype(bf).astype(np.float32)

    npa_ = np.zeros((128, HG, 8), np.float32)
    npb_ = np.zeros((128, HG, 8), np.float32)
    for hg in range(HG):
        blk_hi = n_hi[hg * 128 : (hg + 1) * 128]
        blk_lo = n_lo[hg * 128 : (hg + 1) * 128]
        for rep in (0, 4):
            npa_[:, hg, rep + 0 : rep + 2] = blk_hi
            npa_[:, hg, rep + 2 : rep + 4] = blk_lo
            npb_[:, hg, rep + 0 : rep + 2] = blk_hi
    npa_ = np.ascontiguousarray(npa_.reshape(128, HG * 8).astype(bf))
    npb_ = np.ascontiguousarray(npb_.reshape(128, HG * 8).astype(bf))

    mpa_ = np.zeros((8, H), np.float32)
    mpb_ = np.zeros((8, H), np.float32)
    for k in range(4):
        mpa_[k] = m_hi[:, k % 2]
        mpa_[k + 4] = m_lo[:, k % 2]
        mpb_[k] = m_hi[:, k % 2]
    mpa_ = np.ascontiguousarray(mpa_.astype(bf))
    mpb_ = np.ascontiguousarray(mpb_.astype(bf))

    in_maps = []
    for k in range(NCORES):
        xs = x[k * BL : (k + 1) * BL]          # [BL, T, D]
        xtc = xs.transpose(2, 1, 0)            # [D, T, BL]
        xpad = np.zeros((128, TPAD, BL), np.float32)
        xpad[:, W:, :] = xtc
        in_maps.append(
            {
                "xt": np.ascontiguousarray(xpad.reshape(128, TPAD * BL)),
                "isb": isb,
                "msb": msb,
                "nsb": nsb,
                "npa": npa_,
                "npb": npb_,
                "mpa": mpa_,
                "mpb": mpb_,
            }
        )
    return in_maps


def assemble_output(results):
    L, S, CB, F, SL, TPAD = _derived()
    out = np.empty((B, T, H), np.float32)
    for k in range(NCORES):
        arr = results[k]["outk"].reshape(128, L, HG, C, BL)
        # h[b, c*L + j, hg*128 + p] = arr[p, j, hg, c, b]
        shard = arr.transpose(4, 3, 1, 2, 0).reshape(BL, T, H)
        out[k * BL : (k + 1) * BL] = shard
    return out


def kernel(x, m, n, I, _trace=False):
    nc = _get_nc()
    in_maps = prepare_inputs(x, m, n, I)
    res = run_bass_kernel_spmd(nc, in_maps, list(range(NCORES)), trace=_trace)
    out = assemble_output(res.results)
    if _trace:
        kernel.last_results = res
    return out


# revision 8
# speedup vs baseline: 1.8457x; 1.2409x over previous
"""Trainium2 Bass kernel for nn_LowRankRNN.

Math:  h_{t} = 0.9*h_{t-1} + 0.1*tanh(h_{t-1}) @ (n m^T) + 0.1*xp_t,
       xp_t = x_t @ I^T   (per batch row; sequential over t, B rows independent)

Strategy:
  - Data-parallel over batch: 8 cores x 4 rows each.
  - Time-sharding within each core: C chunks of L=T/C steps; each chunk
    starts W warmup steps early from h=0 (zero-padded x makes chunk 0 exact).
    The recurrence Jacobian has spectral radius ~0.91, so the warmup error
    after W=192 steps is ~3e-8 relative -- below fp32 roundoff.
  - Per serial slot tau, all C chunks advance together: state tile
    [128 partitions = h%128, F = (hg, c, b)] with hg = h//128 (4 groups),
    c = chunk, b = local batch row.
  - Per slot: ACT tanh -> 4 PE matmuls (contract H: v = tanh(h) @ n, rank 2)
    -> DVE copy psum->sbuf -> 4 PE matmuls (expand: g = v @ (0.1 m)^T),
    accumulating onto a PSUM bank pre-staged with e = 0.1*x_t@I^T by bulk
    matmuls -> one fused DVE scalar_tensor_tensor: h' = 0.9*h + psum(e+g).
"""

import sys

sys.path.insert(0, "/opt/trn_rl_repo")

import numpy as np

from concourse import bass, bacc, mybir
from concourse.tile import TileContext
from concourse.bass_utils import run_bass_kernel_spmd

# ---- problem constants (hardcoded; kernel.py must be self-contained) ----
B, T, D, H, R = 32, 2048, 128, 512, 2
ALPHA = 0.1
DECAY = 1.0 - ALPHA  # 0.9
NCORES = 8
BL = B // NCORES  # 4 batch rows per core

# ---- kernel tuning parameters (defaults; overridable via set_config) ----
C = 16            # time chunks per core
W = 160           # warmup steps (error ~1e-6, at fp32 noise level)
HG = H // 128     # 4 h-groups
PSUM_COLS = 512
F32 = mybir.dt.float32
BF16 = mybir.dt.bfloat16


def _derived():
    L = T // C
    S = L + W
    CB = C * BL
    F = HG * CB
    SL = max(1, PSUM_COLS // F)
    TPAD = T + W
    return L, S, CB, F, SL, TPAD


def set_config(c=None, w=None):
    global C, W, _NC_CACHE
    if c is not None:
        C = c
    if w is not None:
        W = w
    _NC_CACHE = None


def build_nc():
    L, S, CB, F, SL, TPAD = _derived()
    nc = bacc.Bacc()

    xt = nc.declare_dram_parameter("xt", [128, TPAD * BL], F32, isOutput=False)
    isb = nc.declare_dram_parameter("isb", [128, H], F32, isOutput=False)
    msb = nc.declare_dram_parameter("msb", [2, H], F32, isOutput=False)
    nsb = nc.declare_dram_parameter("nsb", [128, HG * R], F32, isOutput=False)
    npa = nc.declare_dram_parameter("npa", [128, HG * 8], BF16, isOutput=False)
    npb = nc.declare_dram_parameter("npb", [128, HG * 8], BF16, isOutput=False)
    mpa = nc.declare_dram_parameter("mpa", [8, H], BF16, isOutput=False)
    mpb = nc.declare_dram_parameter("mpb", [8, H], BF16, isOutput=False)
    outk = nc.declare_dram_parameter("outk", [128, L * F], F32, isOutput=True)

    AF = mybir.ActivationFunctionType
    OP = mybir.AluOpType

    with TileContext(nc) as tc:
        with (
            tc.tile_pool(name="const", bufs=1) as constp,
            tc.tile_pool(name="thp", bufs=3) as thp,
            tc.tile_pool(name="vtp", bufs=3) as vtp,
            tc.tile_pool(name="hstate", bufs=8) as hp,
            tc.tile_pool(name="egp", bufs=6, space="PSUM") as egp,
            tc.tile_pool(name="pvp", bufs=2, space="PSUM") as pvp,
        ):
            xt_sb = constp.tile([128, TPAD * BL], F32, tag="xt")
            isb_sb = constp.tile([128, H], F32, tag="isb")
            msb_sb = constp.tile([2, H], F32, tag="msb")
            nsb_sb = constp.tile([128, HG * R], F32, tag="nsb")
            npa_sb = constp.tile([128, HG * 8], BF16, tag="npa")
            npb_sb = constp.tile([128, HG * 8], BF16, tag="npb")
            mpa_sb = constp.tile([8, H], BF16, tag="mpa")
            mpb_sb = constp.tile([8, H], BF16, tag="mpb")
            nc.sync.dma_start(out=xt_sb[:, :], in_=xt[:, :])
            nc.sync.dma_start(out=isb_sb[:, :], in_=isb[:, :])
            nc.sync.dma_start(out=msb_sb[:, :], in_=msb[:, :])
            nc.sync.dma_start(out=nsb_sb[:, :], in_=nsb[:, :])
            nc.sync.dma_start(out=npa_sb[:, :], in_=npa[:, :])
            nc.sync.dma_start(out=npb_sb[:, :], in_=npb[:, :])
            nc.sync.dma_start(out=mpa_sb[:, :], in_=mpa[:, :])
            nc.sync.dma_start(out=mpb_sb[:, :], in_=mpb[:, :])
            # Collapse the many per-DMA-queue semaphores into one barrier so
            # downstream matmuls don't exceed the ISA sync-wait slot limit.
            tc.strict_bb_all_engine_barrier()

            xt_pitch = xt_sb.ap[0][0]  # per-partition pitch in elements

            s_prev = hp.tile([128, F], F32, tag="h")
            nc.vector.memset(s_prev[:, :], 0.0)

            eg = None
            egr = None
            for tau in range(S):
                sl = tau % SL
                if sl == 0:
                    # stage e = 0.1 * x_t @ I^T for the next SL slots into a
                    # fresh psum bank; one matmul per h-group, free dims
                    # (slot, chunk, batch) with overlapping chunk windows.
                    eg = egp.tile([128, PSUM_COLS], F32, tag="eg")
                    egr = eg.rearrange(
                        "p (s g c b) -> p s g c b", s=SL, g=HG, c=C, b=BL
                    )
                    rhs = bass.AP(
                        xt_sb.tensor,
                        xt_sb.offset + tau * BL,
                        [[xt_pitch, 128], [BL, SL], [L * BL, C], [1, BL]],
                    )
                    for hg in range(HG):
                        # start=True clears the whole psum bank, so only the
                        # first matmul of the bank may set it.
                        nc.tensor.matmul(
                            egr[:, :, hg, :, :],
                            isb_sb[:, hg * 128 : (hg + 1) * 128],
                            rhs,
                            start=(hg == 0),
                            stop=False,
                        )

                # th = tanh(h)
                th = thp.tile([128, F], F32, tag="th")
                nc.scalar.activation(th[:, :], s_prev[:, :], AF.Tanh)
                th_hi = thp.tile([128, F], BF16, tag="th_hi")
                nc.vector.tensor_copy(th_hi[:, :], th[:, :])
                th_lo = thp.tile([128, F], BF16, tag="th_lo")
                nc.vector.tensor_tensor(
                    th_lo[:, :], th[:, :], th_hi[:, :], OP.subtract
                )

                # v = tanh(h) @ n : contract H over 4 groups into psum [2, CB]
                pv = pvp.tile([8, CB], F32, tag="pv")
                for hg in range(HG):
                    nc.tensor.matmul(
                        pv[:, :],
                        npa_sb[:, hg * 8 : (hg + 1) * 8],
                        th_hi[:, hg * CB : (hg + 1) * CB],
                        start=(hg == 0),
                        stop=False,
                    )
                for hg in range(HG):
                    nc.tensor.matmul(
                        pv[:, :],
                        npb_sb[:, hg * 8 : (hg + 1) * 8],
                        th_lo[:, hg * CB : (hg + 1) * CB],
                        start=False,
                        stop=(hg == HG - 1),
                    )

                vt_hi = vtp.tile([8, CB], BF16, tag="vt_hi")
                nc.vector.tensor_copy(vt_hi[:, :], pv[:, :])
                vt_lo = vtp.tile([8, CB], BF16, tag="vt_lo")
                nc.vector.tensor_tensor(
                    vt_lo[:, :], pv[:, :], vt_hi[:, :], OP.subtract
                )

                # g = v @ (0.1 m)^T accumulated onto the staged e bank
                for hg in range(HG):
                    nc.tensor.matmul(
                        egr[:, sl, hg, :, :],
                        mpa_sb[:, hg * 128 : (hg + 1) * 128],
                        vt_hi[:, :],
                        start=False,
                        stop=False,
                    )
                    nc.tensor.matmul(
                        egr[:, sl, hg, :, :],
                        mpb_sb[:, hg * 128 : (hg + 1) * 128],
                        vt_lo[:, :],
                        start=False,
                        stop=True,
                    )

                # h' = 0.9*h + (e + g)
                s_new = hp.tile([128, F], F32, tag="h")
                nc.vector.scalar_tensor_tensor(
                    s_new[:, :],
                    s_prev[:, :],
                    DECAY,
                    eg[:, sl * F : (sl + 1) * F],
                    OP.mult,
                    OP.add,
                )

                if tau >= W:
                    j = tau - W
                    nc.sync.dma_start(
                        out=outk[:, j * F : (j + 1) * F], in_=s_new[:, :]
                    )
                s_prev = s_new

    nc.finalize()
    return nc


_NC_CACHE = None


def _get_nc():
    global _NC_CACHE
    if _NC_CACHE is None:
        _NC_CACHE = build_nc()
    return _NC_CACHE


def prepare_inputs(x, m, n, I):
    """Build the per-core input maps (host-side layout transforms)."""
    L, S, CB, F, SL, TPAD = _derived()
    x = np.asarray(x, dtype=np.float32)
    m = np.asarray(m, dtype=np.float32)
    n = np.asarray(n, dtype=np.float32)
    I = np.asarray(I, dtype=np.float32)

    isb = np.ascontiguousarray((ALPHA * I).T)  # [128, H]
    msb = np.ascontiguousarray((ALPHA * m).T)  # [2, H]
    nsb = np.ascontiguousarray(
        n.reshape(HG, 128, R).transpose(1, 0, 2).reshape(128, HG * R)
    )  # [128, (hg, r)]

    import ml_dtypes
    bf = ml_dtypes.bfloat16
    n_hi = n.astype(bf).astype(np.float32)
    n_lo = (n - n_hi).astype(bf).astype(np.float32)
    m01 = (ALPHA * m).astype(np.float32)
    m_hi = m01.astype(bf).astype(np.float32)
    m_lo = (m01 - m_hi).astype(bf).astype(np.float32)

    npa_ = np.zeros((128, HG, 8), np.float32)
    npb_ = np.zeros((128, HG, 8), np.float32)
    for hg in range(HG):
        blk_hi = n_hi[hg * 128 : (hg + 1) * 128]
        blk_lo = n_lo[hg * 128 : (hg + 1) * 128]
        for rep in (0, 4):
            npa_[:, hg, rep + 0 : rep + 2] = blk_hi
            npa_[:, hg, rep + 2 : rep + 4] = blk_lo
            npb_[:, hg, rep + 0 : rep + 2] = blk_hi
    npa_ = np.ascontiguousarray(npa_.reshape(128, HG * 8).astype(bf))
    npb_ = np.ascontiguousarray(npb_.reshape(128, HG * 8).astype(bf))

    mpa_ = np.zeros((8, H), np.float32)
    mpb_ = np.zeros((8, H), np.float32)
    for k in range(4):
        mpa_[k] = m_hi[:, k % 2]
        mpa_[k + 4] = m_lo[:, k % 2]
        mpb_[k] = m_hi[:, k % 2]
    mpa_ = np.ascontiguousarray(mpa_.astype(bf))
    mpb_ = np.ascontiguousarray(mpb_.astype(bf))

    in_maps = []
    for k in range(NCORES):
        xs = x[k * BL : (k + 1) * BL]          # [BL, T, D]
        xtc = xs.transpose(2, 1, 0)            # [D, T, BL]
        xpad = np.zeros((128, TPAD, BL), np.float32)
        xpad[:, W:, :] = xtc
        in_maps.append(
            {
                "xt": np.ascontiguousarray(xpad.reshape(128, TPAD * BL)),
                "isb": isb,
                "msb": msb,
                "nsb": nsb,
                "npa": npa_,
                "npb": npb_,
                "mpa": mpa_,
                "mpb": mpb_,
            }
        )
    return in_maps


def assemble_output(results):
    L, S, CB, F, SL, TPAD = _derived()
    out = np.empty((B, T, H), np.float32)
    for k in range(NCORES):
        arr = results[k]["outk"].reshape(128, L, HG, C, BL)
        # h[b, c*L + j, hg*128 + p] = arr[p, j, hg, c, b]
        shard = arr.transpose(4, 3, 1, 2, 0).reshape(BL, T, H)
        out[k * BL : (k + 1) * BL] = shard
    return out


def kernel(x, m, n, I, _trace=False):
    nc = _get_nc()
    in_maps = prepare_inputs(x, m, n, I)
    res = run_bass_kernel_spmd(nc, in_maps, list(range(NCORES)), trace=_trace)
    out = assemble_output(res.results)
    if _trace:
        kernel.last_results = res
    return out
